# revision 25
# baseline (speedup 1.0000x reference)
"""Trainium2 Bass kernel for nn_Attention_5480378270188.

Single-layer attention: q/k/v linear projections (torch Linear convention),
scores = q @ k^T (no 1/sqrt(d) scale), additive -1e9 mask, softmax over keys,
out = weights @ v.

Shapes (hardcoded): B=8, N=M=2048, D_MODEL=D_K=D_V=1024, fp32 inputs.

Sharding: data-parallel over batch - core b computes batch element b.
mask / weights are replicated to all 8 cores. No collectives.

Algebraic restructure (exact math):
- scores = q @ k^T = (x_q Wq^T + bq)(x_k Wk^T + bk)^T
         = x_q (Wq^T Wk) x_k^T + row-const + col-term + const.
  A = Wq^T Wk is computed on the HOST (weights are tiny and shared across
  batch) and shipped as fp16; the k-projection disappears from the device
  entirely along with Wq/Wk loads and transposes. The col-term bq^T Wk x_k^T
  folds into the qa bias: qa = x_q A + (Wk^T bq). The row-const and const
  terms cancel exactly in softmax.
- bv is applied on the host: softmax rows sum to 1, so W @ (v+bv) = W@v + bv.

On-device dtypes: all TensorE operands fp16 (full PE rate), fp32 PSUM
accumulation, softmax in fp32, fp16 output (upcast on host).

Queue discipline (load-bearing): DRAM-sourced X-bar transpose DMAs
serialize globally against other DRAM-side plain DMAs with ~3-10us dead
time per mode alternation (measured), regardless of queue. So phase A is
ONE uninterrupted X-bar stream on the sync queue - activations AND
weights: the host ships A^T and Wv row-major fp16 so the X-bar lands
them directly in the d-major SBUF layouts the PE wants. The mask (int8,
not transposable) is the only plain load; it rides the scalar queue
whose issue point is naturally serialized behind phase A's PSUM->SBUF
copies, so its transfers run in the quiet window after the last phase-A
X-bar and before the first phase-B X-bar. SBUF-sourced X-bars (phase B's
w16 transposes) do not conflict with plain DMAs - proven by the
baseline - so phase B keeps output DMAs on scalar and the gpsimd engine
does only compute (mask-bias tiles), never SWDGE pumping.

Structure:
- Phase A: qa = x_q @ A and v = x_v @ Wv^T projections; all operand
  transposes via the X-bar stream, PE does matmuls only.
- Phase B (block-pipelined): scores matmuls -> mask-add into SBUF (frees
  PSUM banks early) + chunk maxes -> exp in 1024-wide halves (ACT, fused
  row-sum accum) -> X-bar transpose per half -> PV in two 512-col passes
  (second pass overlaps the first pass's scale+output DMA) -> reciprocal
  scaling -> fp16 output DMA on the scalar queue. Mask bias tiles are
  built one block ahead on the otherwise-idle gpsimd engine, and
  softmax(blk) is emitted before PV(blk-1) so the last block's softmax
  hides behind the previous block's PV matmuls.
"""

import sys

for _p in ("/opt/trn_rl_repo", "/opt/pypackages"):
    if _p not in sys.path:
        sys.path.insert(0, _p)

from contextlib import ExitStack

import numpy as np

import concourse.bass as bass
import concourse.tile as tile
from concourse import bacc, mybir
from concourse.bass import ds, ts
from concourse.bass_utils import run_bass_kernel_spmd

P = 128
B = 8
N = 2048  # queries
M = 2048  # keys
D = 1024  # d_model (= contraction dim for scores after the A-fold)
DV = 1024  # value dim
F = 512  # matmul moving free dim
DT = mybir.dt.float16
F32 = mybir.dt.float32
I8 = mybir.dt.int8

NEG = -1.0e9

N_BLOCKS = N // P  # 16
M_BLOCKS = M // P  # 16
D_O = D // P  # 8
N_MEGA = N // F  # 4 query mega-blocks (512 rows)
M_GRP = M // F  # 4 value groups (512 rows)
SC_CHUNKS = M // F  # 4 score chunks per row-block
PV_CHUNKS = DV // F  # 2


def build():
    nc = bacc.Bacc("TRN2", target_bir_lowering=False, debug=False)

    q16_e = nc.dram_tensor("q16", [N, D], DT, kind="ExternalInput").ap()
    k16_e = nc.dram_tensor("k16", [M, D], DT, kind="ExternalInput").ap()
    v16_e = nc.dram_tensor("v16", [M, D], DT, kind="ExternalInput").ap()
    mask8_e = nc.dram_tensor("mask8", [N, M], I8, kind="ExternalInput").ap()
    A16T_e = nc.dram_tensor("A16T", [D, D], DT, kind="ExternalInput").ap()
    Wv16_e = nc.dram_tensor("Wv16", [DV, D], DT, kind="ExternalInput").ap()
    # u reshaped (8,128) + zero-padded to (16,128) so it X-bar-loads as
    # [P, 16] - keeps phase A free of plain DMAs entirely
    u16_e = nc.dram_tensor("u16", [16, P], DT, kind="ExternalInput").ap()
    out_e = nc.dram_tensor("out", [N, DV], DT, kind="ExternalOutput").ap()

    with tile.TileContext(nc) as tc, ExitStack() as ctx:
        const = ctx.enter_context(tc.tile_pool(name="const", bufs=1))
        persist = ctx.enter_context(tc.tile_pool(name="persist", bufs=1))

        u_sb = const.tile([P, 16], DT, tag="u")
        nc.sync.dma_start(u_sb[:], u16_e[:, :], transpose=True)

        # persistent fp16 operands for the attention matmuls
        kT_sb = persist.tile([P, D_O, M], DT, tag="kT")  # [d_i, d_o, m]
        qaT_sb = persist.tile([P, D_O, N], DT, tag="qaT")  # [j_i, j_o, n]
        v_sb = persist.tile([P, M_BLOCKS, DV], DT, tag="v")  # [m_i, m_o, dv]
        mask8_sb = persist.tile([P, N_BLOCKS, M], I8, tag="mask8")

        # ---------------- Phase A: transposes + projections ----------------
        with (
            tc.tile_pool(name="phW", bufs=1) as pw,
            tc.tile_pool(name="phT", bufs=3) as pact,
            tc.tile_pool(name="psA", bufs=4, space="PSUM") as psA,
        ):
            # A^T and Wv ride the X-bar stream like every other transpose:
            # X-bar of A^T row-block jo lands A[:, jo-block] as [i_i, i_o, j],
            # X-bar of Wv row-blocks lands Wv^T as [d_i, d_o, dv]. A is kept
            # as 8 per-jo tiles so proj jo=0 starts after a single A X-bar
            # instead of all eight.
            A_jo = [
                pw.tile([P, D_O, P], DT, tag=f"A{jo}", name=f"A_{jo}")
                for jo in range(D_O)
            ]
            WvT_sb = pw.tile([P, D_O, DV], DT, tag="WvT")

            def xbar(dst_sb, src_e, blk):
                nc.sync.dma_start(
                    dst_sb[:, :, ds(blk * P, P)],
                    src_e[ds(blk * P, P), :],
                    transpose=True,
                )

            def load_group(src_e, g):
                """512 rows of src -> fresh [P, D_O, F] d-major tile."""
                dst = pact.tile([P, D_O, F], DT, tag="actT", name=f"actT_{g}")
                for b in range(4):
                    nc.sync.dma_start(
                        dst[:, :, ds(b * P, P)],
                        src_e[ds(g * F + b * P, P), :],
                        transpose=True,
                    )
                return dst

            def proj_q_jo(g, qTt, jo):
                ps = psA.tile([P, F], F32, tag="ps_a")
                for io in range(D_O):
                    nc.tensor.matmul(
                        ps[:],
                        A_jo[jo][:, io, :],
                        qTt[:, io, :],
                        start=(io == 0),
                        stop=(io == D_O - 1),
                    )
                nc.scalar.add(
                    qaT_sb[:, jo, ds(g * F, F)], ps[:], u_sb[:, jo : jo + 1]
                )

            def proj_q(g, qTt):
                for jo in range(D_O):
                    proj_q_jo(g, qTt, jo)

            def proj_v(grp, vtT):
                for r in range(4):
                    mo = grp * 4 + r
                    pss = [
                        psA.tile([P, F], F32, tag="ps_a", name=f"ps_v_{c}")
                        for c in range(PV_CHUNKS)
                    ]
                    for io in range(D_O):
                        for c in range(PV_CHUNKS):
                            nc.tensor.matmul(
                                pss[c][:],
                                vtT[:, io, ds(r * P, P)],
                                WvT_sb[:, io, ts(c, F)],
                                start=(io == 0),
                                stop=(io == D_O - 1),
                            )
                    for c in range(PV_CHUNKS):
                        # ACT-engine copies keep the scalar instruction stream
                        # busy so the mask DMA issues (emitted last) fire in
                        # the quiet window after the phase-A X-bars
                        nc.scalar.copy(v_sb[:, mo, ts(c, F)], pss[c][:])

            # the one uninterrupted X-bar stream, interleaved with projections;
            # X-bars are ordered so each consumer stays a few slots behind
            # the stream (k8-15 are not needed until phase B and go last)
            qT0 = load_group(q16_e, 0)
            for jo in range(D_O):
                nc.sync.dma_start(
                    A_jo[jo][:, :, :], A16T_e[ds(jo * P, P), :], transpose=True
                )
                proj_q_jo(0, qT0, jo)
            q_tiles = [load_group(q16_e, 1), load_group(q16_e, 2)]
            for kb in range(4):
                xbar(kT_sb, k16_e, kb)
            proj_q(1, q_tiles[0])
            q_tiles.append(load_group(q16_e, 3))
            for kb in range(4, 8):
                xbar(kT_sb, k16_e, kb)
            proj_q(2, q_tiles[1])
            for db in range(D_O):
                xbar(WvT_sb, Wv16_e, db)
            v_tiles = [load_group(v16_e, 0), load_group(v16_e, 1)]
            proj_q(3, q_tiles[2])
            v_tiles.append(load_group(v16_e, 2))
            proj_v(0, v_tiles[0])
            v_tiles.append(load_group(v16_e, 3))
            proj_v(1, v_tiles[1])
            proj_v(2, v_tiles[2])
            for kb in range(8, 16):
                xbar(kT_sb, k16_e, kb)
            proj_v(3, v_tiles[3])

            # mask: the only plain DMA. The wait hint pins it into the quiet
            # window after the last phase-A X-bar and before phase B's first
            # w16 X-bar (the scheduler otherwise hoists dep-free DMAs to t=0,
            # and plain DRAM reads alternating with DRAM X-bars cost ~5-10us
            # of dead time per switch).
            with tc.tile_wait_until(0.105):
                for blk in range(N_BLOCKS):
                    nc.scalar.dma_start(
                        mask8_sb[:, blk, :], mask8_e[ds(blk * P, P), :]
                    )

        # ---------------- Phase B: attention blocks ----------------
        with (
            tc.tile_pool(name="mainp", bufs=2) as mp,
            tc.tile_pool(name="psSC", bufs=4, space="PSUM") as psSC,
            tc.tile_pool(name="psPV", bufs=2, space="PSUM") as psPV,
        ):
            state = {}
            btiles = {}

            def build_btile(blk):
                # additive mask bias: mask8 * 1e9 - 1e9 -> {0, -1e9}; built
                # on the otherwise-idle gpsimd engine, one block ahead
                bt = mp.tile([P, M], F32, tag="maskbias", name=f"bt_{blk}")
                nc.gpsimd.tensor_scalar(
                    bt[:],
                    mask8_sb[:, blk, :],
                    -NEG,
                    NEG,
                    mybir.AluOpType.mult,
                    mybir.AluOpType.add,
                )
                btiles[blk] = bt

            def scores_part(blk, chunks, soft):
                if "btile" not in soft:
                    soft["btile"] = btiles.pop(blk)
                    soft["scf"] = mp.tile([P, M], F32, tag="scf", name=f"scf_{blk}")
                    # one small tile holds stats[0:4], sums[4:6], negmax[6],
                    # rsum[7], rinv[8] (fewer slots -> fewer exit barriers)
                    soft["sm"] = mp.tile([P, 9], F32, tag="smalls", name=f"sm_{blk}")
                btile, scf, sm = soft["btile"], soft["scf"], soft["sm"]
                stats = sm[:, 0:SC_CHUNKS]

                # scores: qaT block tile stationary, reused across all 4 chunks
                for mc in chunks:
                    ps = psSC.tile([P, F], F32, tag="ps_sc", name=f"ps_sc_{mc}")
                    for jo in range(D_O):
                        nc.tensor.matmul(
                            ps[:],
                            qaT_sb[:, jo, ds(blk * P, P)],
                            kT_sb[:, jo, ts(mc, F)],
                            start=(jo == 0),
                            stop=(jo == D_O - 1),
                        )
                    # mask-add PSUM -> SBUF frees the PSUM bank early
                    nc.vector.tensor_add(
                        scf[:, ts(mc, F)], ps[:], btile[:, ts(mc, F)]
                    )
                    nc.vector.reduce_max(
                        stats[:, mc : mc + 1],
                        scf[:, ts(mc, F)],
                        axis=mybir.AxisListType.X,
                    )
            def softmax_tail(blk, soft):
                scf, sm = soft["scf"], soft["sm"]
                stats = sm[:, 0:SC_CHUNKS]
                sums = sm[:, 4:6]
                negmax = sm[:, 6:7]
                rsum = sm[:, 7:8]
                rinv = sm[:, 8:9]
                w16 = mp.tile([P, M], DT, tag="w16")
                if blk + 1 < N_BLOCKS:
                    build_btile(blk + 1)
                nc.vector.reduce_max(
                    negmax[:], stats[:], axis=mybir.AxisListType.X, negate=True
                )

                # exp in 1024-wide halves with fused row-sum accumulation;
                # X-bar transpose of each half as soon as it is ready
                wT = mp.tile([P, M_BLOCKS, P], DT, tag="wT")
                for h in range(2):
                    nc.scalar.activation(
                        w16[:, ds(h * 1024, 1024)],
                        scf[:, ds(h * 1024, 1024)],
                        mybir.ActivationFunctionType.Exp,
                        bias=negmax[:, 0:1],
                        scale=1.0,
                        accum_out=sums[:, h : h + 1],
                    )
                    nc.sync.dma_start(
                        wT[:, ds(h * 8, 8), :],
                        w16[:, ds(h * 1024, 1024)],
                        transpose=True,
                    )
                nc.vector.reduce_sum(rsum[:], sums[:], axis=mybir.AxisListType.X)
                nc.vector.reciprocal(rinv[:], rsum[:])
                state[blk] = (wT, rinv)

            pvs = {}

            def pv_pass(blk, c):
                wT, _ = state[blk]
                if c == 0:
                    pvs[blk] = psPV.tile(
                        [P, PV_CHUNKS, F], F32, tag="ps_pv", name=f"pv_{blk}"
                    )
                pv = pvs[blk]
                for mo in range(M_BLOCKS):
                    nc.tensor.matmul(
                        pv[:, c, :],
                        wT[:, mo, :],
                        v_sb[:, mo, ts(c, F)],
                        start=(mo == 0),
                        stop=(mo == M_BLOCKS - 1),
                    )

            def scale_out(blk):
                _, rinv = state.pop(blk)
                pv = pvs.pop(blk)
                outt = mp.tile([P, DV], DT, tag="outt")
                for c in range(PV_CHUNKS):
                    nc.vector.tensor_scalar_mul(
                        outt[:, ts(c, F)], pv[:, c, :], rinv[:, 0:1]
                    )
                    nc.scalar.dma_start(
                        out_e[ds(blk * P, P), ts(c, F)], outt[:, ts(c, F)]
                    )

            def pv_out(blk):
                # two passes over c so c=0's scale+DMA overlaps c=1's matmuls
                pv_pass(blk, 0)
                pv_pass(blk, 1)
                scale_out(blk)

            build_btile(0)
            for blk in range(N_BLOCKS - 1):
                soft = {}
                scores_part(blk, range(SC_CHUNKS), soft)
                softmax_tail(blk, soft)
                if blk > 0:
                    pv_out(blk - 1)
            # last block: PV(14)'s passes are emitted BETWEEN its score
            # chunks so the PE keeps matmul work in flight while softmax(15)
            # runs on DVE/ACT/X-bar (otherwise the scheduler runs PV(14)
            # early and the PE idles ~4us before PV(15))
            soft = {}
            scores_part(N_BLOCKS - 1, range(2), soft)
            pv_pass(N_BLOCKS - 2, 0)
            scores_part(N_BLOCKS - 1, range(2, SC_CHUNKS), soft)
            softmax_tail(N_BLOCKS - 1, soft)
            pv_pass(N_BLOCKS - 2, 1)
            scale_out(N_BLOCKS - 2)
            pv_out(N_BLOCKS - 1)

    nc.compile()
    return nc


_CACHE = {}


def _get_nc():
    if "nc" not in _CACHE:
        _CACHE["nc"] = build()
    return _CACHE["nc"]


def run(inputs, trace=False, trace_kwargs=None):
    nc = _get_nc()
    q16 = np.ascontiguousarray(np.asarray(inputs["querys"]).astype(np.float16))
    k16 = np.ascontiguousarray(np.asarray(inputs["keys"]).astype(np.float16))
    v16 = np.ascontiguousarray(np.asarray(inputs["values"]).astype(np.float16))
    mask8 = np.ascontiguousarray(np.asarray(inputs["mask"]).astype(np.int8))
    Wq = np.asarray(inputs["Wq"], dtype=np.float32)
    Wk = np.asarray(inputs["Wk"], dtype=np.float32)
    Wv = np.asarray(inputs["Wv"], dtype=np.float32)
    bq = np.asarray(inputs["bq"], dtype=np.float32)
    # A = Wq^T Wk folds the k-projection away; u = Wk^T bq is the exact
    # surviving bias term (row-constant terms cancel in softmax). A is
    # shipped TRANSPOSED and Wv as-is: the X-bar load un-transposes them.
    A16T = np.ascontiguousarray((Wk.T @ Wq).astype(np.float16))
    u16 = np.zeros((16, P), dtype=np.float16)
    u16[:D_O] = (Wk.T @ bq).astype(np.float16).reshape(D_O, P)
    Wv16 = np.ascontiguousarray(Wv.astype(np.float16))
    shared = {"mask8": mask8, "A16T": A16T, "Wv16": Wv16, "u16": u16}
    in_maps = [
        {
            "q16": q16[b],
            "k16": k16[b],
            "v16": v16[b],
            **shared,
        }
        for b in range(B)
    ]
    res = run_bass_kernel_spmd(
        nc,
        in_maps,
        list(range(B)),
        trace=trace,
        **(trace_kwargs or {}),
    )
    out = np.stack([res.results[b]["out"] for b in range(B)]).astype(np.float32)
    # bv folded in on the host: softmax rows sum to 1, so W @ (v + bv) = W @ v + bv
    out += np.asarray(inputs["bv"], dtype=np.float32)[None, None, :]
    return out, res


def kernel(**inputs) -> np.ndarray:
    out, _ = run(inputs, trace=False)
    return out


if __name__ == "__main__":
    nc = _get_nc()
    print("built + compiled OK")


# revision 26
# speedup vs baseline: 1.0193x; 1.0193x over previous
"""Trainium2 Bass kernel for nn_Attention_5480378270188.

Single-layer attention: q/k/v linear projections (torch Linear convention),
scores = q @ k^T (no 1/sqrt(d) scale), additive -1e9 mask, softmax over keys,
out = weights @ v.

Shapes (hardcoded): B=8, N=M=2048, D_MODEL=D_K=D_V=1024, fp32 inputs.

Sharding: data-parallel over batch - core b computes batch element b.
mask / weights are replicated to all 8 cores. No collectives.

Algebraic restructure (exact math):
- scores = q @ k^T = (x_q Wq^T + bq)(x_k Wk^T + bk)^T
         = x_q (Wq^T Wk) x_k^T + row-const + col-term + const.
  A = Wq^T Wk is computed on the HOST (weights are tiny and shared across
  batch) and shipped as fp16; the k-projection disappears from the device
  entirely along with Wq/Wk loads and transposes. The col-term bq^T Wk x_k^T
  folds into the qa bias: qa = x_q A + (Wk^T bq). The row-const and const
  terms cancel exactly in softmax.
- bv is applied on the host: softmax rows sum to 1, so W @ (v+bv) = W@v + bv.

On-device dtypes: all TensorE operands fp16 (full PE rate), fp32 PSUM
accumulation, softmax in fp32, fp16 output (upcast on host).

Queue discipline (load-bearing): DRAM-sourced X-bar transpose DMAs
serialize globally against other DRAM-side plain DMAs with ~3-10us dead
time per mode alternation (measured), regardless of queue. So phase A is
ONE uninterrupted X-bar stream on the sync queue - activations AND
weights: the host ships A^T and Wv row-major fp16 so the X-bar lands
them directly in the d-major SBUF layouts the PE wants. The mask (int8,
not transposable) is the only plain load; it rides the scalar queue
whose issue point is naturally serialized behind phase A's PSUM->SBUF
copies, so its transfers run in the quiet window after the last phase-A
X-bar and before the first phase-B X-bar. SBUF-sourced X-bars (phase B's
w16 transposes) do not conflict with plain DMAs - proven by the
baseline - so phase B keeps output DMAs on scalar and the gpsimd engine
does only compute (mask-bias tiles), never SWDGE pumping.

Structure:
- Phase A: qa = x_q @ A and v = x_v @ Wv^T projections; all operand
  transposes via the X-bar stream, PE does matmuls only.
- Phase B (block-pipelined): scores matmuls -> mask-add into SBUF (frees
  PSUM banks early) + chunk maxes -> exp in 1024-wide halves (ACT, fused
  row-sum accum) -> X-bar transpose per half -> PV in two 512-col passes
  (second pass overlaps the first pass's scale+output DMA) -> reciprocal
  scaling -> fp16 output DMA on the scalar queue. Mask bias tiles are
  built one block ahead on the otherwise-idle gpsimd engine, and
  softmax(blk) is emitted before PV(blk-1) so the last block's softmax
  hides behind the previous block's PV matmuls.
"""

import sys

for _p in ("/opt/trn_rl_repo", "/opt/pypackages"):
    if _p not in sys.path:
        sys.path.insert(0, _p)

from contextlib import ExitStack

import numpy as np

import concourse.bass as bass
import concourse.tile as tile
from concourse import bacc, mybir
from concourse.bass import ds, ts
from concourse.bass_utils import run_bass_kernel_spmd

P = 128
B = 8
N = 2048  # queries
M = 2048  # keys
D = 1024  # d_model (= contraction dim for scores after the A-fold)
DV = 1024  # value dim
F = 512  # matmul moving free dim
DT = mybir.dt.float16
F32 = mybir.dt.float32
I8 = mybir.dt.int8

NEG = -1.0e9

N_BLOCKS = N // P  # 16
M_BLOCKS = M // P  # 16
D_O = D // P  # 8
N_MEGA = N // F  # 4 query mega-blocks (512 rows)
M_GRP = M // F  # 4 value groups (512 rows)
SC_CHUNKS = M // F  # 4 score chunks per row-block
PV_CHUNKS = DV // F  # 2


def build():
    nc = bacc.Bacc("TRN2", target_bir_lowering=False, debug=False)

    q16_e = nc.dram_tensor("q16", [N, D], DT, kind="ExternalInput").ap()
    k16_e = nc.dram_tensor("k16", [M, D], DT, kind="ExternalInput").ap()
    v16_e = nc.dram_tensor("v16", [M, D], DT, kind="ExternalInput").ap()
    mask8_e = nc.dram_tensor("mask8", [N, M], I8, kind="ExternalInput").ap()
    A16T_e = nc.dram_tensor("A16T", [D, D], DT, kind="ExternalInput").ap()
    Wv16_e = nc.dram_tensor("Wv16", [DV, D], DT, kind="ExternalInput").ap()
    # u reshaped (8,128) + zero-padded to (16,128) so it X-bar-loads as
    # [P, 16] - keeps phase A free of plain DMAs entirely
    u16_e = nc.dram_tensor("u16", [16, P], DT, kind="ExternalInput").ap()
    out_e = nc.dram_tensor("out", [N, DV], DT, kind="ExternalOutput").ap()

    with tile.TileContext(nc) as tc, ExitStack() as ctx:
        const = ctx.enter_context(tc.tile_pool(name="const", bufs=1))
        persist = ctx.enter_context(tc.tile_pool(name="persist", bufs=1))

        u_sb = const.tile([P, 16], DT, tag="u")
        nc.sync.dma_start(u_sb[:], u16_e[:, :], transpose=True)

        # persistent fp16 operands for the attention matmuls
        kT_sb = persist.tile([P, D_O, M], DT, tag="kT")  # [d_i, d_o, m]
        qaT_sb = persist.tile([P, D_O, N], DT, tag="qaT")  # [j_i, j_o, n]
        v_sb = persist.tile([P, M_BLOCKS, DV], DT, tag="v")  # [m_i, m_o, dv]
        mask8_sb = persist.tile([P, N_BLOCKS, M], I8, tag="mask8")

        # ---------------- Phase A: transposes + projections ----------------
        with (
            tc.tile_pool(name="phW", bufs=1) as pw,
            tc.tile_pool(name="phT", bufs=3) as pact,
            tc.tile_pool(name="psA", bufs=4, space="PSUM") as psA,
        ):
            # A^T and Wv ride the X-bar stream like every other transpose:
            # X-bar of A^T row-block jo lands A[:, jo-block] as [i_i, i_o, j],
            # X-bar of Wv row-blocks lands Wv^T as [d_i, d_o, dv]. A is kept
            # as 8 per-jo tiles so proj jo=0 starts after a single A X-bar
            # instead of all eight.
            A_jo = [
                pw.tile([P, D_O, P], DT, tag=f"A{jo}", name=f"A_{jo}")
                for jo in range(D_O)
            ]
            WvT_sb = pw.tile([P, D_O, DV], DT, tag="WvT")

            def xbar(dst_sb, src_e, blk):
                nc.sync.dma_start(
                    dst_sb[:, :, ds(blk * P, P)],
                    src_e[ds(blk * P, P), :],
                    transpose=True,
                )

            def load_group(src_e, g):
                """512 rows of src -> fresh [P, D_O, F] d-major tile."""
                dst = pact.tile([P, D_O, F], DT, tag="actT", name=f"actT_{g}")
                for b in range(4):
                    nc.sync.dma_start(
                        dst[:, :, ds(b * P, P)],
                        src_e[ds(g * F + b * P, P), :],
                        transpose=True,
                    )
                return dst

            def proj_q_jo(g, qTt, jo):
                ps = psA.tile([P, F], F32, tag="ps_a")
                for io in range(D_O):
                    nc.tensor.matmul(
                        ps[:],
                        A_jo[jo][:, io, :],
                        qTt[:, io, :],
                        start=(io == 0),
                        stop=(io == D_O - 1),
                    )
                nc.scalar.add(
                    qaT_sb[:, jo, ds(g * F, F)], ps[:], u_sb[:, jo : jo + 1]
                )

            def proj_q(g, qTt):
                for jo in range(D_O):
                    proj_q_jo(g, qTt, jo)

            def proj_v(grp, vtT):
                for r in range(4):
                    mo = grp * 4 + r
                    pss = [
                        psA.tile([P, F], F32, tag="ps_a", name=f"ps_v_{c}")
                        for c in range(PV_CHUNKS)
                    ]
                    for io in range(D_O):
                        for c in range(PV_CHUNKS):
                            nc.tensor.matmul(
                                pss[c][:],
                                vtT[:, io, ds(r * P, P)],
                                WvT_sb[:, io, ts(c, F)],
                                start=(io == 0),
                                stop=(io == D_O - 1),
                            )
                    for c in range(PV_CHUNKS):
                        # ACT-engine copies keep the scalar instruction stream
                        # busy so the mask DMA issues (emitted last) fire in
                        # the quiet window after the phase-A X-bars
                        nc.scalar.copy(v_sb[:, mo, ts(c, F)], pss[c][:])

            # the one uninterrupted X-bar stream, interleaved with projections;
            # X-bars are ordered so each consumer stays a few slots behind
            # the stream (k8-15 are not needed until phase B and go last)
            qT0 = load_group(q16_e, 0)
            for jo in range(D_O):
                nc.sync.dma_start(
                    A_jo[jo][:, :, :], A16T_e[ds(jo * P, P), :], transpose=True
                )
                proj_q_jo(0, qT0, jo)
            q_tiles = [load_group(q16_e, 1), load_group(q16_e, 2)]
            for kb in range(4):
                xbar(kT_sb, k16_e, kb)
            proj_q(1, q_tiles[0])
            q_tiles.append(load_group(q16_e, 3))
            for kb in range(4, 8):
                xbar(kT_sb, k16_e, kb)
            proj_q(2, q_tiles[1])
            for db in range(D_O):
                xbar(WvT_sb, Wv16_e, db)
            v_tiles = [load_group(v16_e, 0), load_group(v16_e, 1)]
            proj_q(3, q_tiles[2])
            v_tiles.append(load_group(v16_e, 2))
            proj_v(0, v_tiles[0])
            v_tiles.append(load_group(v16_e, 3))
            proj_v(1, v_tiles[1])
            proj_v(2, v_tiles[2])
            for kb in range(8, 16):
                xbar(kT_sb, k16_e, kb)
            proj_v(3, v_tiles[3])

            # mask: the only plain DMA. The wait hint pins it into the quiet
            # window after the last phase-A X-bar and before phase B's first
            # w16 X-bar (the scheduler otherwise hoists dep-free DMAs to t=0,
            # and plain DRAM reads alternating with DRAM X-bars cost ~5-10us
            # of dead time per switch).
            with tc.tile_wait_until(0.105):
                for blk in range(N_BLOCKS):
                    nc.scalar.dma_start(
                        mask8_sb[:, blk, :], mask8_e[ds(blk * P, P), :]
                    )

        # ---------------- Phase B: attention blocks ----------------
        with (
            tc.tile_pool(name="mainp", bufs=2) as mp,
            tc.tile_pool(name="psSC", bufs=4, space="PSUM") as psSC,
            tc.tile_pool(name="psPV", bufs=2, space="PSUM") as psPV,
        ):
            state = {}
            btiles = {}

            def build_btile(blk):
                # additive mask bias: mask8 * 1e9 - 1e9 -> {0, -1e9}; built
                # on the otherwise-idle gpsimd engine, one block ahead
                bt = mp.tile([P, M], F32, tag="maskbias", name=f"bt_{blk}")
                nc.gpsimd.tensor_scalar(
                    bt[:],
                    mask8_sb[:, blk, :],
                    -NEG,
                    NEG,
                    mybir.AluOpType.mult,
                    mybir.AluOpType.add,
                )
                btiles[blk] = bt

            def scores_softmax(blk):
                btile = btiles.pop(blk)
                scf = mp.tile([P, M], F32, tag="scf")
                # one small tile holds stats[0:4], sums[4:6], negmax[6],
                # rsum[7], rinv[8] (fewer slots -> fewer exit barriers)
                sm = mp.tile([P, 9], F32, tag="smalls")
                stats = sm[:, 0:SC_CHUNKS]
                sums = sm[:, 4:6]
                negmax = sm[:, 6:7]
                rsum = sm[:, 7:8]
                rinv = sm[:, 8:9]
                w16 = mp.tile([P, M], DT, tag="w16")

                # scores: qaT block tile stationary, reused across all 4 chunks
                for mc in range(SC_CHUNKS):
                    ps = psSC.tile([P, F], F32, tag="ps_sc", name=f"ps_sc_{mc}")
                    for jo in range(D_O):
                        nc.tensor.matmul(
                            ps[:],
                            qaT_sb[:, jo, ds(blk * P, P)],
                            kT_sb[:, jo, ts(mc, F)],
                            start=(jo == 0),
                            stop=(jo == D_O - 1),
                        )
                    # mask-add PSUM -> SBUF frees the PSUM bank early
                    nc.vector.tensor_add(
                        scf[:, ts(mc, F)], ps[:], btile[:, ts(mc, F)]
                    )
                    nc.vector.reduce_max(
                        stats[:, mc : mc + 1],
                        scf[:, ts(mc, F)],
                        axis=mybir.AxisListType.X,
                    )
                if blk + 1 < N_BLOCKS:
                    build_btile(blk + 1)
                nc.vector.reduce_max(
                    negmax[:], stats[:], axis=mybir.AxisListType.X, negate=True
                )

                # exp in 1024-wide halves with fused row-sum accumulation;
                # X-bar transpose of each half as soon as it is ready
                wT = mp.tile([P, M_BLOCKS, P], DT, tag="wT")
                for h in range(2):
                    nc.scalar.activation(
                        w16[:, ds(h * 1024, 1024)],
                        scf[:, ds(h * 1024, 1024)],
                        mybir.ActivationFunctionType.Exp,
                        bias=negmax[:, 0:1],
                        scale=1.0,
                        accum_out=sums[:, h : h + 1],
                    )
                    nc.sync.dma_start(
                        wT[:, ds(h * 8, 8), :],
                        w16[:, ds(h * 1024, 1024)],
                        transpose=True,
                    )
                nc.vector.reduce_sum(rsum[:], sums[:], axis=mybir.AxisListType.X)
                nc.vector.reciprocal(rinv[:], rsum[:])
                state[blk] = (wT, rinv)

            def pv_out(blk):
                wT, rinv = state.pop(blk)
                pv = psPV.tile([P, PV_CHUNKS, F], F32, tag="ps_pv")
                # two passes over c so c=0's scale+DMA overlaps c=1's matmuls
                for c in range(PV_CHUNKS):
                    for mo in range(M_BLOCKS):
                        nc.tensor.matmul(
                            pv[:, c, :],
                            wT[:, mo, :],
                            v_sb[:, mo, ts(c, F)],
                            start=(mo == 0),
                            stop=(mo == M_BLOCKS - 1),
                        )
                outt = mp.tile([P, DV], DT, tag="outt")
                for c in range(PV_CHUNKS):
                    nc.vector.tensor_scalar_mul(
                        outt[:, ts(c, F)], pv[:, c, :], rinv[:, 0:1]
                    )
                    nc.scalar.dma_start(
                        out_e[ds(blk * P, P), ts(c, F)], outt[:, ts(c, F)]
                    )

            build_btile(0)
            for blk in range(N_BLOCKS):
                scores_softmax(blk)
                if blk > 0:
                    pv_out(blk - 1)
            pv_out(N_BLOCKS - 1)

    nc.compile()
    return nc


_CACHE = {}


def _get_nc():
    if "nc" not in _CACHE:
        _CACHE["nc"] = build()
    return _CACHE["nc"]


def run(inputs, trace=False, trace_kwargs=None):
    nc = _get_nc()
    q16 = np.ascontiguousarray(np.asarray(inputs["querys"]).astype(np.float16))
    k16 = np.ascontiguousarray(np.asarray(inputs["keys"]).astype(np.float16))
    v16 = np.ascontiguousarray(np.asarray(inputs["values"]).astype(np.float16))
    mask8 = np.ascontiguousarray(np.asarray(inputs["mask"]).astype(np.int8))
    Wq = np.asarray(inputs["Wq"], dtype=np.float32)
    Wk = np.asarray(inputs["Wk"], dtype=np.float32)
    Wv = np.asarray(inputs["Wv"], dtype=np.float32)
    bq = np.asarray(inputs["bq"], dtype=np.float32)
    # A = Wq^T Wk folds the k-projection away; u = Wk^T bq is the exact
    # surviving bias term (row-constant terms cancel in softmax). A is
    # shipped TRANSPOSED and Wv as-is: the X-bar load un-transposes them.
    A16T = np.ascontiguousarray((Wk.T @ Wq).astype(np.float16))
    u16 = np.zeros((16, P), dtype=np.float16)
    u16[:D_O] = (Wk.T @ bq).astype(np.float16).reshape(D_O, P)
    Wv16 = np.ascontiguousarray(Wv.astype(np.float16))
    shared = {"mask8": mask8, "A16T": A16T, "Wv16": Wv16, "u16": u16}
    in_maps = [
        {
            "q16": q16[b],
            "k16": k16[b],
            "v16": v16[b],
            **shared,
        }
        for b in range(B)
    ]
    res = run_bass_kernel_spmd(
        nc,
        in_maps,
        list(range(B)),
        trace=trace,
        **(trace_kwargs or {}),
    )
    out = np.stack([res.results[b]["out"] for b in range(B)]).astype(np.float32)
    # bv folded in on the host: softmax rows sum to 1, so W @ (v + bv) = W @ v + bv
    out += np.asarray(inputs["bv"], dtype=np.float32)[None, None, :]
    return out, res


def kernel(**inputs) -> np.ndarray:
    out, _ = run(inputs, trace=False)
    return out


if __name__ == "__main__":
    nc = _get_nc()
    print("built + compiled OK")


# revision 30
# speedup vs baseline: 1.2172x; 1.1941x over previous
"""Trainium2 Bass kernel for nn_Attention_5480378270188.

Single-layer attention: q/k/v linear projections (torch Linear convention),
scores = q @ k^T (no 1/sqrt(d) scale), additive -1e9 mask, softmax over keys,
out = weights @ v.

Shapes (hardcoded): B=8, N=M=2048, D_MODEL=D_K=D_V=1024, fp32 inputs.

Sharding: data-parallel over batch - core b computes batch element b.
mask / weights are replicated to all 8 cores. No collectives.

Algebraic restructure (exact math):
- scores = q @ k^T = (x_q Wq^T + bq)(x_k Wk^T + bk)^T
         = x_q (Wq^T Wk) x_k^T + row-const + col-term + const.
  A = Wq^T Wk is computed on the HOST (weights are tiny and shared across
  batch) and shipped as fp16; the k-projection disappears from the device
  entirely along with Wq/Wk loads and transposes. The col-term bq^T Wk x_k^T
  folds into the qa bias: qa = x_q A + (Wk^T bq). The row-const and const
  terms cancel exactly in softmax.
- bv is applied on the host: softmax rows sum to 1, so W @ (v+bv) = W@v + bv.

On-device dtypes: all TensorE operands fp16 (full PE rate), fp32 PSUM
accumulation, softmax in fp32, fp16 output (upcast on host).

Queue discipline (load-bearing): DRAM-sourced X-bar transpose DMAs
serialize globally against other DRAM-side plain DMAs with ~3-10us dead
time per mode alternation (measured), regardless of queue. So phase A is
ONE uninterrupted X-bar stream on the sync queue - activations AND
weights: the host ships A^T and Wv row-major fp16 so the X-bar lands
them directly in the d-major SBUF layouts the PE wants. The mask (int8,
not transposable) is the only plain load; it rides the scalar queue
whose issue point is naturally serialized behind phase A's PSUM->SBUF
copies, so its transfers run in the quiet window after the last phase-A
X-bar and before the first phase-B X-bar. SBUF-sourced X-bars (phase B's
w16 transposes) do not conflict with plain DMAs - proven by the
baseline - so phase B keeps output DMAs on scalar and the gpsimd engine
does only compute (mask-bias tiles), never SWDGE pumping.

Structure:
- Phase A: qa = x_q @ A and v = x_v @ Wv^T projections; all operand
  transposes via the X-bar stream, PE does matmuls only.
- Phase B (block-pipelined): scores matmuls -> mask-add into SBUF (frees
  PSUM banks early) + chunk maxes -> exp in 1024-wide halves (ACT, fused
  row-sum accum) -> X-bar transpose per half -> PV in two 512-col passes
  (second pass overlaps the first pass's scale+output DMA) -> reciprocal
  scaling -> fp16 output DMA on the scalar queue. Mask bias tiles are
  built one block ahead on the otherwise-idle gpsimd engine, and
  softmax(blk) is emitted before PV(blk-1) so the last block's softmax
  hides behind the previous block's PV matmuls.
"""

import sys

for _p in ("/opt/trn_rl_repo", "/opt/pypackages"):
    if _p not in sys.path:
        sys.path.insert(0, _p)

from contextlib import ExitStack

import numpy as np

import concourse.bass as bass
import concourse.tile as tile
from concourse import bacc, mybir
from concourse.bass import ds, ts
from concourse.bass_utils import run_bass_kernel_spmd

P = 128
B = 8
N = 2048  # queries
M = 2048  # keys
D = 1024  # d_model (= contraction dim for scores after the A-fold)
DV = 1024  # value dim
F = 512  # matmul moving free dim
DT = mybir.dt.float16
F32 = mybir.dt.float32
I8 = mybir.dt.int8

NEG = -1.0e9

N_BLOCKS = N // P  # 16
M_BLOCKS = M // P  # 16
D_O = D // P  # 8
N_MEGA = N // F  # 4 query mega-blocks (512 rows)
M_GRP = M // F  # 4 value groups (512 rows)
SC_CHUNKS = M // F  # 4 score chunks per row-block
PV_CHUNKS = DV // F  # 2


def build():
    nc = bacc.Bacc("TRN2", target_bir_lowering=False, debug=False)

    q16_e = nc.dram_tensor("q16", [N, D], DT, kind="ExternalInput").ap()
    k16_e = nc.dram_tensor("k16", [M, D], DT, kind="ExternalInput").ap()
    v16_e = nc.dram_tensor("v16", [M, D], DT, kind="ExternalInput").ap()
    mask8_e = nc.dram_tensor("mask8", [N, M], I8, kind="ExternalInput").ap()
    A16T_e = nc.dram_tensor("A16T", [D, D], DT, kind="ExternalInput").ap()
    Wv16_e = nc.dram_tensor("Wv16", [DV, D], DT, kind="ExternalInput").ap()
    # u reshaped (8,128) + zero-padded to (16,128) so it X-bar-loads as
    # [P, 16] - keeps phase A free of plain DMAs entirely
    u16_e = nc.dram_tensor("u16", [16, P], DT, kind="ExternalInput").ap()
    out_e = nc.dram_tensor("out", [N, DV], DT, kind="ExternalOutput").ap()

    with tile.TileContext(nc) as tc, ExitStack() as ctx:
        const = ctx.enter_context(tc.tile_pool(name="const", bufs=1))
        persist = ctx.enter_context(tc.tile_pool(name="persist", bufs=1))
        # one PSUM pool set for the whole kernel (phase B's score tiles share
        # the projection pool's slots): no PSUM pool release/alloc barrier -
        # and no PE p-state dip - at the phase A -> B boundary
        psA = ctx.enter_context(tc.tile_pool(name="psA", bufs=4, space="PSUM"))
        psPV = ctx.enter_context(tc.tile_pool(name="psPV", bufs=2, space="PSUM"))

        u_sb = const.tile([P, 16], DT, tag="u")
        nc.sync.dma_start(u_sb[:], u16_e[:, :], transpose=True)

        # persistent fp16 operands for the attention matmuls
        kT_sb = persist.tile([P, D_O, M], DT, tag="kT")  # [d_i, d_o, m]
        qaT_sb = persist.tile([P, D_O, N], DT, tag="qaT")  # [j_i, j_o, n]
        v_sb = persist.tile([P, M_BLOCKS, DV], DT, tag="v")  # [m_i, m_o, dv]
        mask8_sb = persist.tile([P, N_BLOCKS, M], I8, tag="mask8")

        # ---------------- Phase A: transposes + projections ----------------
        with (
            tc.tile_pool(name="phW", bufs=1) as pw,
            tc.tile_pool(name="phT", bufs=3) as pact,
        ):
            # A^T and Wv ride the X-bar stream like every other transpose:
            # X-bar of A^T row-block jo lands A[:, jo-block] as [i_i, i_o, j],
            # X-bar of Wv row-blocks lands Wv^T as [d_i, d_o, dv]. A is kept
            # as 8 per-jo tiles so proj jo=0 starts after a single A X-bar
            # instead of all eight.
            A_jo = [
                pw.tile([P, D_O, P], DT, tag=f"A{jo}", name=f"A_{jo}")
                for jo in range(D_O)
            ]
            WvT_sb = pw.tile([P, D_O, DV], DT, tag="WvT")

            def xbar(dst_sb, src_e, blk):
                nc.sync.dma_start(
                    dst_sb[:, :, ds(blk * P, P)],
                    src_e[ds(blk * P, P), :],
                    transpose=True,
                )

            def load_group(src_e, g):
                """512 rows of src -> fresh [P, D_O, F] d-major tile."""
                dst = pact.tile([P, D_O, F], DT, tag="actT", name=f"actT_{g}")
                for b in range(4):
                    nc.sync.dma_start(
                        dst[:, :, ds(b * P, P)],
                        src_e[ds(g * F + b * P, P), :],
                        transpose=True,
                    )
                return dst

            def proj_q_jo(g, qTt, jo):
                ps = psA.tile([P, F], F32, tag="ps_a")
                for io in range(D_O):
                    nc.tensor.matmul(
                        ps[:],
                        A_jo[jo][:, io, :],
                        qTt[:, io, :],
                        start=(io == 0),
                        stop=(io == D_O - 1),
                    )
                nc.scalar.add(
                    qaT_sb[:, jo, ds(g * F, F)], ps[:], u_sb[:, jo : jo + 1]
                )

            def proj_q(g, qTt):
                for jo in range(D_O):
                    proj_q_jo(g, qTt, jo)

            def proj_v(grp, vtT):
                for r in range(4):
                    mo = grp * 4 + r
                    pss = [
                        psA.tile([P, F], F32, tag="ps_a", name=f"ps_v_{c}")
                        for c in range(PV_CHUNKS)
                    ]
                    for io in range(D_O):
                        for c in range(PV_CHUNKS):
                            nc.tensor.matmul(
                                pss[c][:],
                                vtT[:, io, ds(r * P, P)],
                                WvT_sb[:, io, ts(c, F)],
                                start=(io == 0),
                                stop=(io == D_O - 1),
                            )
                    for c in range(PV_CHUNKS):
                        # ACT-engine copies keep the scalar instruction stream
                        # busy so the mask DMA issues (emitted last) fire in
                        # the quiet window after the phase-A X-bars
                        nc.scalar.copy(v_sb[:, mo, ts(c, F)], pss[c][:])

            # the one uninterrupted X-bar stream, interleaved with projections;
            # X-bars are ordered so each consumer stays a few slots behind
            # the stream (k8-15 are not needed until phase B and go last)
            qT0 = load_group(q16_e, 0)
            for jo in range(D_O):
                nc.sync.dma_start(
                    A_jo[jo][:, :, :], A16T_e[ds(jo * P, P), :], transpose=True
                )
                proj_q_jo(0, qT0, jo)
            q_tiles = [load_group(q16_e, 1), load_group(q16_e, 2)]
            for kb in range(4):
                xbar(kT_sb, k16_e, kb)
            proj_q(1, q_tiles[0])
            q_tiles.append(load_group(q16_e, 3))
            for kb in range(4, 8):
                xbar(kT_sb, k16_e, kb)
            proj_q(2, q_tiles[1])
            for db in range(D_O):
                xbar(WvT_sb, Wv16_e, db)
            v_tiles = [load_group(v16_e, 0), load_group(v16_e, 1)]
            proj_q(3, q_tiles[2])
            v_tiles.append(load_group(v16_e, 2))
            proj_v(0, v_tiles[0])
            v_tiles.append(load_group(v16_e, 3))
            proj_v(1, v_tiles[1])
            proj_v(2, v_tiles[2])
            for kb in range(8, 16):
                xbar(kT_sb, k16_e, kb)
            proj_v(3, v_tiles[3])

            # mask: the only plain DMA. The wait hint pins it into the quiet
            # window after the last phase-A X-bar and before phase B's first
            # w16 X-bar (the scheduler otherwise hoists dep-free DMAs to t=0,
            # and plain DRAM reads alternating with DRAM X-bars cost ~5-10us
            # of dead time per switch).
            with tc.tile_wait_until(0.105):
                for blk in range(N_BLOCKS):
                    nc.scalar.dma_start(
                        mask8_sb[:, blk, :], mask8_e[ds(blk * P, P), :]
                    )

        # ---------------- Phase B: attention blocks ----------------
        with tc.tile_pool(name="mainp", bufs=2) as mp:
            state = {}
            btiles = {}

            def build_btile(blk):
                # additive mask bias: mask8 * 1e9 - 1e9 -> {0, -1e9}; built
                # on the otherwise-idle gpsimd engine, one block ahead
                bt = mp.tile([P, M], F32, tag="maskbias", name=f"bt_{blk}")
                nc.gpsimd.tensor_scalar(
                    bt[:],
                    mask8_sb[:, blk, :],
                    -NEG,
                    NEG,
                    mybir.AluOpType.mult,
                    mybir.AluOpType.add,
                )
                btiles[blk] = bt

            def scores_softmax(blk):
                btile = btiles.pop(blk)
                scf = mp.tile([P, M], F32, tag="scf")
                # one small tile holds stats[0:4], sums[4:6], negmax[6],
                # rsum[7], rinv[8] (fewer slots -> fewer exit barriers)
                sm = mp.tile([P, 9], F32, tag="smalls")
                stats = sm[:, 0:SC_CHUNKS]
                sums = sm[:, 4:6]
                negmax = sm[:, 6:7]
                rsum = sm[:, 7:8]
                rinv = sm[:, 8:9]
                w16 = mp.tile([P, M], DT, tag="w16")

                # scores: qaT block tile stationary, reused across all 4 chunks
                for mc in range(SC_CHUNKS):
                    ps = psA.tile([P, F], F32, tag="ps_a", name=f"ps_sc_{mc}")
                    for jo in range(D_O):
                        nc.tensor.matmul(
                            ps[:],
                            qaT_sb[:, jo, ds(blk * P, P)],
                            kT_sb[:, jo, ts(mc, F)],
                            start=(jo == 0),
                            stop=(jo == D_O - 1),
                        )
                    # mask-add PSUM -> SBUF frees the PSUM bank early
                    nc.vector.tensor_add(
                        scf[:, ts(mc, F)], ps[:], btile[:, ts(mc, F)]
                    )
                    nc.vector.reduce_max(
                        stats[:, mc : mc + 1],
                        scf[:, ts(mc, F)],
                        axis=mybir.AxisListType.X,
                    )
                if blk + 1 < N_BLOCKS:
                    build_btile(blk + 1)
                nc.vector.reduce_max(
                    negmax[:], stats[:], axis=mybir.AxisListType.X, negate=True
                )

                # exp in 1024-wide halves with fused row-sum accumulation;
                # X-bar transpose of each half as soon as it is ready
                wT = mp.tile([P, M_BLOCKS, P], DT, tag="wT")
                for h in range(2):
                    nc.scalar.activation(
                        w16[:, ds(h * 1024, 1024)],
                        scf[:, ds(h * 1024, 1024)],
                        mybir.ActivationFunctionType.Exp,
                        bias=negmax[:, 0:1],
                        scale=1.0,
                        accum_out=sums[:, h : h + 1],
                    )
                    nc.sync.dma_start(
                        wT[:, ds(h * 8, 8), :],
                        w16[:, ds(h * 1024, 1024)],
                        transpose=True,
                    )
                nc.vector.reduce_sum(rsum[:], sums[:], axis=mybir.AxisListType.X)
                nc.vector.reciprocal(rinv[:], rsum[:])
                state[blk] = (wT, rinv)

            def pv_out(blk):
                wT, rinv = state.pop(blk)
                pv = psPV.tile([P, PV_CHUNKS, F], F32, tag="ps_pv")
                # two passes over c so c=0's scale+DMA overlaps c=1's matmuls
                for c in range(PV_CHUNKS):
                    for mo in range(M_BLOCKS):
                        nc.tensor.matmul(
                            pv[:, c, :],
                            wT[:, mo, :],
                            v_sb[:, mo, ts(c, F)],
                            start=(mo == 0),
                            stop=(mo == M_BLOCKS - 1),
                        )
                outt = mp.tile([P, DV], DT, tag="outt")
                for c in range(PV_CHUNKS):
                    nc.vector.tensor_scalar_mul(
                        outt[:, ts(c, F)], pv[:, c, :], rinv[:, 0:1]
                    )
                    nc.scalar.dma_start(
                        out_e[ds(blk * P, P), ts(c, F)], outt[:, ts(c, F)]
                    )

            build_btile(0)
            for blk in range(N_BLOCKS):
                scores_softmax(blk)
                if blk > 0:
                    pv_out(blk - 1)
            pv_out(N_BLOCKS - 1)

    nc.compile()
    return nc


_CACHE = {}


def _get_nc():
    if "nc" not in _CACHE:
        _CACHE["nc"] = build()
    return _CACHE["nc"]


def run(inputs, trace=False, trace_kwargs=None):
    nc = _get_nc()
    q16 = np.ascontiguousarray(np.asarray(inputs["querys"]).astype(np.float16))
    k16 = np.ascontiguousarray(np.asarray(inputs["keys"]).astype(np.float16))
    v16 = np.ascontiguousarray(np.asarray(inputs["values"]).astype(np.float16))
    mask8 = np.ascontiguousarray(np.asarray(inputs["mask"]).astype(np.int8))
    Wq = np.asarray(inputs["Wq"], dtype=np.float32)
    Wk = np.asarray(inputs["Wk"], dtype=np.float32)
    Wv = np.asarray(inputs["Wv"], dtype=np.float32)
    bq = np.asarray(inputs["bq"], dtype=np.float32)
    # A = Wq^T Wk folds the k-projection away; u = Wk^T bq is the exact
    # surviving bias term (row-constant terms cancel in softmax). A is
    # shipped TRANSPOSED and Wv as-is: the X-bar load un-transposes them.
    A16T = np.ascontiguousarray((Wk.T @ Wq).astype(np.float16))
    u16 = np.zeros((16, P), dtype=np.float16)
    u16[:D_O] = (Wk.T @ bq).astype(np.float16).reshape(D_O, P)
    Wv16 = np.ascontiguousarray(Wv.astype(np.float16))
    shared = {"mask8": mask8, "A16T": A16T, "Wv16": Wv16, "u16": u16}
    in_maps = [
        {
            "q16": q16[b],
            "k16": k16[b],
            "v16": v16[b],
            **shared,
        }
        for b in range(B)
    ]
    res = run_bass_kernel_spmd(
        nc,
        in_maps,
        list(range(B)),
        trace=trace,
        **(trace_kwargs or {}),
    )
    out = np.stack([res.results[b]["out"] for b in range(B)]).astype(np.float32)
    # bv folded in on the host: softmax rows sum to 1, so W @ (v + bv) = W @ v + bv
    out += np.asarray(inputs["bv"], dtype=np.float32)[None, None, :]
    return out, res


def kernel(**inputs) -> np.ndarray:
    out, _ = run(inputs, trace=False)
    return out


if __name__ == "__main__":
    nc = _get_nc()
    print("built + compiled OK")
